# revision 101
# baseline (speedup 1.0000x reference)
"""DeepSeek-V3 MLA attention kernel for 8 Trainium2 NeuronCores.

Problem: nn_DeepSeekV3_1Attention (B=2, S=2048, D=2048, H=16, NOPE=128,
ROPE=64, VD=128, QL=KVL=512), fp32 reference, causal.

Sharding: data-parallel over batch (2 groups of 4 cores) x tensor-parallel
over heads (4 heads per core). Each core computes its batch's shared
projections redundantly, runs MLA attention for its 4 heads, and produces
a partial out-projection (its heads' rows of out_w). Host sums the 4
partials per batch.

Phase 1 fuses the low-rank q path on the host (W_qn = q_down @ q_up_nope,
W_qr = q_down @ q_up_rope) so q_nope^T / q_rope^T / c_kv^T / k_rope^T all
come from one weight-stationary pass over hs^T: each 128-col weight chunk
sweeps all four 512-token blocks with a single Ldweights per lhsT (the PE
sequencer, not the array, limits fp8 DoubleRow throughput otherwise).
Projections stay at the fp8-pair scale (sigma = 16*512); the sigma^2 is
divided out inside the softmax exp, and the 1/sigma for the value path
rides the vabs copies.

Attention keeps sequence on the free dimension (scores^T[k, q]) with
exp(s - 2) in fp16, a multiplicative causal mask, the softmax denominator
as a pairwise fp16 tree on the DVE plus one cross-partition reduce on
Pool, and normalization deferred past the (linear) PV and out-projection.
"""

import numpy as np
import ml_dtypes

NF8 = ml_dtypes.float8_e4m3


def _split8(x, s):
    """x*s ~ hi + lo, both fp8e4m3 (shared power-of-2 scale s)."""
    xs = np.asarray(x, np.float32) * np.float32(s)
    hi = xs.astype(NF8)
    lo = (xs - hi.astype(np.float32)).astype(NF8)
    return np.ascontiguousarray(hi), np.ascontiguousarray(lo)

from concourse import bacc
import concourse.bass as bass
import concourse.bass_isa as bass_isa
import concourse.mybir as mybir
import concourse.tile as tile
from concourse.bass_utils import run_bass_kernel_spmd

F32 = mybir.dt.float32
F32R = mybir.dt.float32r
BF16 = mybir.dt.bfloat16
F16 = mybir.dt.float16
F8 = mybir.dt.float8e4
DR = mybir.MatmulPerfMode.DoubleRow
AF = mybir.ActivationFunctionType

B, S, D = 2, 2048, 2048
H = 16
NOPE, ROPE, VD = 128, 64, 128
QL, KVL = 512, 512
HPC = 4    # heads per core
G = 4      # cores per batch group
SCALE = float(1.0 / np.sqrt(np.float32(NOPE + ROPE)))
SIG = 16.0 * 512.0      # fp8-pair product scale carried by all projections
ESCALE = SCALE / (SIG * SIG)
EXPB = -2.0   # exp bias: e' = exp(s - 2) keeps row sums within fp16 range

ROPE_WAVELENGTH = 10000.0
ROPE_SCALE = 40.0
BETA_FAST, BETA_SLOW = 32.0, 1.0
OLD_CTX = 4096.0
MSCALE = 1.0
PI = 3.14159265358979

NDC = D // 128          # 16 d-chunks
NQLC = QL // 128        # 4 ql chunks
NKVC = KVL // 128       # 4 kv chunks
NKC = S // 128          # 16 key chunks
NQB = S // 512          # 4 query blocks


def _rope_tables():
    j = np.arange(0, ROPE, 2, dtype=np.float32) / ROPE
    freqs = (1.0 / (ROPE_WAVELENGTH ** j)).astype(np.float32)
    wavelengths = 2.0 * PI / freqs
    ramp = np.clip((wavelengths / OLD_CTX - BETA_SLOW) / (BETA_FAST - BETA_SLOW),
                   0.0, 1.0)
    scale = (1.0 - ramp) + ramp * ROPE_SCALE
    inv_freq = freqs / scale
    t = np.arange(S, dtype=np.float32)
    fr = t[:, None] * inv_freq[None, :]
    cos = (np.cos(fr) * MSCALE).astype(np.float32).T        # [32, S]
    sin = (np.sin(fr) * MSCALE).astype(np.float32).T
    cosT = np.concatenate([cos, cos], 0)                    # [64, S]
    sinT = np.concatenate([-sin, sin], 0)
    cos2 = np.ascontiguousarray(np.concatenate([cosT, cosT], 0))   # [128, S]
    sin2 = np.ascontiguousarray(np.concatenate([sinT, sinT], 0))
    return cos2, sin2


def _masks():
    # multiplicative 0/1 masks applied to exp(scores) on the diagonal chunks
    k = np.arange(128)[:, None]
    q = np.arange(512)[None, :]
    ms = []
    for m in range(4):
        allow = (k + m * 128) <= q
        ms.append(np.where(allow, np.float32(1.0), np.float32(0.0)))
    return np.ascontiguousarray(np.stack(ms, axis=1))    # [128, 4, 512]


def build_nc():
    nc = bacc.Bacc("TRN2", target_bir_lowering=False, debug=False,
                   enable_asserts=False, num_devices=8)

    hsTh = nc.dram_tensor("hsTh", [D, S], F8, kind="ExternalInput").ap()
    hsTl = nc.dram_tensor("hsTl", [D, S], F8, kind="ExternalInput").ap()
    wqnh = nc.dram_tensor("wqnh", [D, HPC * NOPE], F8, kind="ExternalInput").ap()
    wqnl = nc.dram_tensor("wqnl", [D, HPC * NOPE], F8, kind="ExternalInput").ap()
    wqrh = nc.dram_tensor("wqrh", [D, HPC * ROPE], F8, kind="ExternalInput").ap()
    wqrl = nc.dram_tensor("wqrl", [D, HPC * ROPE], F8, kind="ExternalInput").ap()
    kvdwh = nc.dram_tensor("kvdwh", [D, KVL], F8, kind="ExternalInput").ap()
    kvdwl = nc.dram_tensor("kvdwl", [D, KVL], F8, kind="ExternalInput").ap()
    krwh = nc.dram_tensor("krwh", [D, 2 * ROPE], F8, kind="ExternalInput").ap()
    krwl = nc.dram_tensor("krwl", [D, 2 * ROPE], F8, kind="ExternalInput").ap()
    wukT = nc.dram_tensor("wukT", [HPC * KVL, NOPE], F32R, kind="ExternalInput").ap()
    wuv4 = nc.dram_tensor("wuv4", [KVL, HPC * VD], F32R, kind="ExternalInput").ap()
    owgh = nc.dram_tensor("owgh", [HPC * VD, D], F8, kind="ExternalInput").ap()
    owgl = nc.dram_tensor("owgl", [HPC * VD, D], F8, kind="ExternalInput").ap()
    cos2d = nc.dram_tensor("cos2d", [2 * ROPE, S], BF16, kind="ExternalInput").ap()
    sin2d = nc.dram_tensor("sin2d", [2 * ROPE, S], BF16, kind="ExternalInput").ap()
    maskd = nc.dram_tensor("maskd", [128, 4, 512], F16, kind="ExternalInput").ap()
    outT = nc.dram_tensor("outT", [D, S], BF16, kind="ExternalOutput").ap()

    hsTh_r = hsTh.rearrange("(c p) s -> p c s", p=128)    # [128, 16, S]
    hsTl_r = hsTl.rearrange("(c p) s -> p c s", p=128)
    wqnh_r = wqnh.rearrange("(c p) q -> p c q", p=128)    # [128, 16, 512]
    wqnl_r = wqnl.rearrange("(c p) q -> p c q", p=128)
    wqrh_r = wqrh.rearrange("(c p) q -> p c q", p=128)    # [128, 16, 256]
    wqrl_r = wqrl.rearrange("(c p) q -> p c q", p=128)
    kvdwh_r = kvdwh.rearrange("(c p) q -> p c q", p=128)
    kvdwl_r = kvdwl.rearrange("(c p) q -> p c q", p=128)
    krwh_r = krwh.rearrange("(c p) q -> p c q", p=128)    # [128, 16, 128]
    krwl_r = krwl.rearrange("(c p) q -> p c q", p=128)
    wukT_r = wukT.rearrange("(c p) n -> p c n", p=128)    # [128, 16, 128]
    wuv4_r = wuv4.rearrange("(c p) v -> p c v", p=128)    # [128, 4, 512]
    owgh_r = owgh.rearrange("(h p) d -> p h d", p=128)    # [128, 4, D]
    owgl_r = owgl.rearrange("(h p) d -> p h d", p=128)

    with tile.TileContext(nc) as tc:
        with tc.tile_pool(name="A", bufs=1) as A:
            # phase-1 outputs, all at scale SIG (= 16*512); sequence on the
            # free dim so every downstream matmul contracts partitions
            c_kvT = A.tile([128, NKVC, S], F32R, tag="c_kvT")
            qnT = A.tile([128, HPC, S], BF16, tag="qnT")
            # two heads stacked per 128 partitions (psum-native layout)
            qrT2 = A.tile([128, HPC // 2, S], BF16, tag="qrT2")
            kr2 = A.tile([128, S], BF16, tag="kr2")     # k_rope duplicated 2x
            # phase-2 weights live here so their DMAs overlap phase 1
            # (a P2-scoped tile would inherit anti-deps on phase-1 SBUF)
            wukT_t = A.tile([128, HPC * NQLC, NOPE], F32R, tag="wukT")
            wuv4_t = A.tile([128, NKVC, HPC * VD], F32R, tag="wuv4")
            masks_t = A.tile([128, 4, 512], F16, tag="masks")

            # -------- phase 1: one weight-stationary pass over hs^T --------
            # 3-term fp8 DoubleRow over the D contraction; each 128-col
            # weight chunk sweeps all 4 s-blocks per Ldweights
            with tc.tile_pool(name="P1", bufs=1) as P1, \
                 tc.tile_pool(name="P1r", bufs=1) as P1r, \
                 tc.tile_pool(name="PS1", bufs=2, space="PSUM") as PS1:
                hsh_t = P1.tile([128, NDC, S], F8, tag="hsh")
                hsl_t = P1.tile([128, NDC, S], F8, tag="hsl")
                wqnh_t = P1.tile([128, NDC, HPC * NOPE], F8, tag="wqnh")
                wqnl_t = P1.tile([128, NDC, HPC * NOPE], F8, tag="wqnl")
                wqrh_t = P1.tile([128, NDC, HPC * ROPE], F8, tag="wqrh")
                wqrl_t = P1.tile([128, NDC, HPC * ROPE], F8, tag="wqrl")
                kvdwh_t = P1.tile([128, NDC, KVL], F8, tag="kvdwh")
                kvdwl_t = P1.tile([128, NDC, KVL], F8, tag="kvdwl")
                krwh_t = P1.tile([128, NDC, 2 * ROPE], F8, tag="krwh")
                krwl_t = P1.tile([128, NDC, 2 * ROPE], F8, tag="krwl")
                cos2_t = P1.tile([2 * ROPE, S], BF16, tag="cos2")
                sin2_t = P1.tile([2 * ROPE, S], BF16, tag="sin2")
                # DMA order tracks first use: a sliver of c_kv weights, two
                # hs chunks (first matmuls start ~4us in), the rest of the
                # weights, then the hs stream (its per-chunk DMA rate matches
                # PE consumption); q-path weights trail (first used 5
                # tile-cols in)
                # first matmul needs only these two transfers
                nc.sync.dma_start(kvdwh_t[:, 0:2, :], kvdwh_r[:, 0:2, :])
                nc.sync.dma_start(hsh_t[:, 0:2, :], hsTh_r[:, 0:2, :])
                nc.sync.dma_start(kvdwl_t[:, 0:2, :], kvdwl_r[:, 0:2, :])
                nc.sync.dma_start(hsl_t[:, 0:2, :], hsTl_r[:, 0:2, :])
                nc.sync.dma_start(kvdwh_t[:, 2:NDC, :], kvdwh_r[:, 2:NDC, :])
                nc.sync.dma_start(kvdwl_t[:, 2:NDC, :], kvdwl_r[:, 2:NDC, :])
                for c in range(2, NDC):
                    nc.sync.dma_start(hsh_t[:, c, :], hsTh_r[:, c, :])
                    nc.sync.dma_start(hsl_t[:, c, :], hsTl_r[:, c, :])
                nc.sync.dma_start(krwh_t[:, :, :], krwh_r[:, :, :])
                nc.sync.dma_start(krwl_t[:, :, :], krwl_r[:, :, :])
                nc.sync.dma_start(cos2_t[:, :], cos2d[:, :])
                nc.sync.dma_start(sin2_t[:, :], sin2d[:, :])
                for t, r in [(wqnh_t, wqnh_r), (wqnl_t, wqnl_r),
                             (wqrh_t, wqrh_r), (wqrl_t, wqrl_r)]:
                    nc.sync.dma_start(t[:, :, :], r[:, :, :])
                nc.sync.dma_start(masks_t[:, :, :], maskd[:, :, :])
                nc.sync.dma_start(wukT_t[:, :, :], wukT_r[:, :, :])
                nc.sync.dma_start(wuv4_t[:, :, :], wuv4_r[:, :, :])

                cp_rot = [0]

                def rot_copy(dst, src):
                    # PSUM->SBUF: DVE/Act only (Pool cannot access PSUM)
                    if cp_rot[0] % 2 == 0:
                        nc.scalar.copy(dst, src)
                    else:
                        nc.vector.tensor_copy(dst, src)
                    cp_rot[0] += 1

                def run_tilecols(group):
                    # dp-major across the group's tile-cols: the first two
                    # run interleaved so ~21us of PE demand rides the
                    # ~28us hs DMA stream instead of ~10us
                    pss = []
                    for _ in group:
                        ps1t = PS1.tile([128, NQB, 512], F32, tag="p1ps")
                        pss.append(ps1t)
                    for dp in range(NDC // 2):
                        d2 = bass.ds(2 * dp, 2)
                        for ci, (wh_t, wl_t, cs, emit) in enumerate(group):
                            terms = [(wh_t, hsh_t), (wl_t, hsh_t),
                                     (wh_t, hsl_t)]
                            for ti, (wt, ht) in enumerate(terms):
                                for sb in range(NQB):
                                    nc.tensor.matmul(
                                        pss[ci][:, sb, :], wt[:, d2, cs],
                                        ht[:, d2, bass.ds(sb * 512, 512)],
                                        start=(dp == 0 and ti == 0),
                                        stop=(dp == NDC // 2 - 1 and ti == 2),
                                        perf_mode=DR)
                    for ci, (wh_t, wl_t, cs, emit) in enumerate(group):
                        emit(pss[ci])

                def emit_copy(dst):
                    def e(ps):
                        rot_copy(dst, ps[:, :, :])
                    return e

                def emit_rope(dst):
                    def e(ps):
                        # rope via swap trick: out = raw*cos + swap(raw)*sin
                        # (sin table carries the sign). One copy drains the
                        # PSUM strip immediately; swaps then run SBUF->SBUF
                        # off the PE-critical path.
                        t1 = P1r.tile([128, NQB * 512], BF16, tag="t1")
                        rot_copy(t1[:, :], ps[:, :, :])
                        sw = P1r.tile([128, NQB * 512], BF16, tag="sw")
                        nc.scalar.copy(sw[0:32, :], t1[32:64, :])
                        nc.vector.tensor_copy(sw[32:64, :], t1[0:32, :])
                        nc.scalar.copy(sw[64:96, :], t1[96:128, :])
                        nc.vector.tensor_copy(sw[96:128, :], t1[64:96, :])
                        nc.vector.tensor_mul(t1[:, :], t1[:, :], cos2_t[:, :])
                        nc.vector.tensor_mul(sw[:, :], sw[:, :], sin2_t[:, :])
                        nc.vector.tensor_add(dst, t1[:, :], sw[:, :])
                    return e

                cols = []
                for qlc in range(NKVC):
                    cols.append((kvdwh_t, kvdwl_t, bass.ts(qlc, 128),
                                 emit_copy(c_kvT[:, qlc, :])))
                cols.append((krwh_t, krwl_t, bass.ds(0, 128),
                             emit_rope(kr2[:, :])))
                cols.append((wqnh_t, wqnl_t, bass.ts(0, 128),
                             emit_copy(qnT[:, 0, :])))
                cols.append((wqrh_t, wqrl_t, bass.ts(0, 128),
                             emit_rope(qrT2[:, 0, :])))
                cols.append((wqnh_t, wqnl_t, bass.ts(1, 128),
                             emit_copy(qnT[:, 1, :])))
                cols.append((wqnh_t, wqnl_t, bass.ts(2, 128),
                             emit_copy(qnT[:, 2, :])))
                cols.append((wqrh_t, wqrl_t, bass.ts(1, 128),
                             emit_rope(qrT2[:, 1, :])))
                cols.append((wqnh_t, wqnl_t, bass.ts(3, 128),
                             emit_copy(qnT[:, 3, :])))
                run_tilecols(cols[0:2])
                for col in cols[2:]:
                    run_tilecols([col])

            # per-head context as residual fp8 pair at scale 16 (the 16
            # rides the vabs copies; phase 3 descales on the host)
            with tc.tile_pool(name="A2", bufs=1) as A2:
                oh_hi = A2.tile([128, HPC, S], F8, tag="oh_hi")
                oh_lo = A2.tile([128, HPC, S], F8, tag="oh_lo")
                # phase-3 weights: DMA overlaps the attention phase
                owgh_t = A2.tile([128, HPC, D], F8, tag="owgh")
                owgl_t = A2.tile([128, HPC, D], F8, tag="owgl")
                for hl in range(HPC):
                    nc.sync.dma_start(owgh_t[:, hl, :], owgh_r[:, hl, :])
                    nc.sync.dma_start(owgl_t[:, hl, :], owgl_r[:, hl, :])

                # -------- phase 2: per-head attention --------
                with tc.tile_pool(name="P2", bufs=1) as P2, \
                     tc.tile_pool(name="P2q2", bufs=2) as P2q2, \
                     tc.tile_pool(name="P2v", bufs=1) as P2v, \
                     tc.tile_pool(name="P2e", bufs=6) as P2e, \
                     tc.tile_pool(name="P2o", bufs=2) as P2o, \
                     tc.tile_pool(name="P2r", bufs=1) as P2r, \
                     tc.tile_pool(name="P2t", bufs=6) as P2t, \
                     tc.tile_pool(name="PSmm", bufs=5, space="PSUM") as PSmm, \
                     tc.tile_pool(name="PSov", bufs=3, space="PSUM") as PSov:
                    biasb = P2.tile([128, 1], F32, tag="expb")
                    nc.vector.memset(biasb[:, :], EXPB)

                    # absorbed values for all 4 heads in one N=512 pass:
                    # vabs = c_kv @ w_uv^T, x(16/SIG) in the copies
                    vabs4 = P2v.tile([128, NKC, HPC * VD], F16, tag="vabs")
                    VSC = 16.0 / SIG
                    for kc in range(NKC):
                        ps4 = PSmm.tile([128, HPC * VD], F32, tag="mm")
                        for kvc in range(NKVC):
                            nc.tensor.matmul(
                                ps4[:, :],
                                c_kvT[:, kvc, bass.ts(kc, 128)],
                                wuv4_t[:, kvc, :],
                                start=(kvc == 0), stop=(kvc == NKVC - 1))
                        if kc % 2 == 0:
                            nc.vector.tensor_scalar_mul(vabs4[:, kc, :],
                                                        ps4[:, :], VSC)
                        else:
                            nc.scalar.activation(vabs4[:, kc, :], ps4[:, :],
                                                 AF.Copy, scale=VSC)

                    self_ka = [None]   # current head's absorbed keys
                    # wide f16 ones: the final pair's row-sum runs as PE
                    # matmuls (broadcast across partitions), shortening the
                    # serial epilogue chain that gates phase 3
                    onesw = P2.tile([128, NOPE], F16, tag="onesw")
                    nc.vector.memset(onesw[:, :], 1.0)

                    def prologue(hl):
                        """absorbed keys k_abs = w_uk_h @ c_kv^T for one head
                        (contracting scores over NOPE=128 instead of KVL)."""
                        kabs = P2q2.tile([128, S], BF16, tag="kabs")
                        for b4 in range(NQB):
                            s4 = bass.ds(b4 * 512, 512)
                            ps3 = PSmm.tile([128, 512], F32, tag="mm")
                            for latc in range(NQLC):
                                nc.tensor.matmul(
                                    ps3[:, :],
                                    wukT_t[:, hl * NQLC + latc, :],
                                    c_kvT[:, latc, s4],
                                    start=(latc == 0), stop=(latc == NQLC - 1))
                            if b4 % 2 == 0:
                                nc.vector.tensor_copy(kabs[:, s4], ps3[:, :])
                            else:
                                nc.scalar.copy(kabs[:, s4], ps3[:, :])
                        self_ka[0] = kabs
                        return kabs

                    # last head runs its big qb=3 block early so that
                    # epilogue hides under the small qb=0 block, shortening
                    # the stall before phase 3's hp1 stages
                    pairs = [(hl, qb) for hl in range(HPC)
                             for qb in (1, 2, 3, 0)]
                    kabs = prologue(0)
                    pending_epi = None   # deferred out_v + normalize
                    e_allocs = [0]       # P2e allocation counter (uninit guard)

                    for idx, (hl, qb) in enumerate(pairs):
                        qs = bass.ds(qb * 512, 512)
                        nkc = 4 * qb + 4
                        kabs = self_ka[0]
                        half = 64 * (hl % 2)

                        ov_ps = PSov.tile([128, 512], F32, tag="ov")
                        # softmax denominator: pairwise fp16 tree on the DVE,
                        # then one cross-partition reduce on Pool
                        levels = [[] for _ in range(6)]
                        last_es = []

                        def tree_push(t, lv=0):
                            if levels[lv]:
                                a = levels[lv].pop()
                                s = P2t.tile([128, 512], F16, tag="tsum")
                                nc.vector.tensor_add(s[:, :], a[:, :], t[:, :])
                                tree_push(s, lv + 1)
                            else:
                                levels[lv].append(t)

                        pends = []   # deferred exp tiles for PE pipelining

                        def flush(pend, ov_ps=ov_ps, nkc=nkc, hl=hl):
                            e, kc, o = pend
                            nc.tensor.matmul(
                                ov_ps[:, o:512],
                                vabs4[:, kc, bass.ds(hl * VD, VD)],
                                e[:, o:512],
                                start=(kc == 0), stop=(kc == nkc - 1))

                        for kc in range(nkc):
                            # diagonal chunks: exact causal width
                            m = kc - 4 * qb
                            o = 0 if m < 0 else m * 128
                            ps_s = PSmm.tile([128, 512], F32, tag="mm")
                            nc.tensor.matmul(
                                ps_s[:, o:512],
                                kabs[:, bass.ts(kc, 128)],
                                qnT[:, hl, bass.ds(qb * 512 + o, 512 - o)],
                                start=True, stop=False)
                            nc.tensor.matmul(
                                ps_s[:, o:512],
                                kr2[half:half + 64, bass.ts(kc, 128)],
                                qrT2[half:half + 64, hl // 2,
                                     bass.ds(qb * 512 + o, 512 - o)],
                                start=False, stop=True)
                            e = P2e.tile([128, 512], F16, tag="exp")
                            if m >= 0 and o > 0 and e_allocs[0] < 12:
                                # first pool rotations: [0:o) is uninit SBUF;
                                # zero it so the full-width mask mul below
                                # never reads garbage bits (NaN-safe on hw)
                                nc.gpsimd.memset(e[:, 0:o], 0.0)
                            e_allocs[0] += 1
                            nc.scalar.activation(e[:, o:512], ps_s[:, o:512],
                                                 AF.Exp, scale=ESCALE,
                                                 bias=biasb[:, :])
                            if m >= 0:
                                # causal mask, full width: also zeroes the
                                # stale [0:o) region for the denominator tree
                                nc.vector.tensor_mul(
                                    e[:, :], e[:, :], masks_t[:, m, :])
                            if idx == len(pairs) - 1:
                                last_es.append(e)
                            else:
                                tree_push(e)
                            if kc == (3 if nkc == 4 else 5) and pending_epi is not None:
                                # previous pair's out_v runs two score-blocks
                                # into this pair, hiding its ctx copy latency
                                pending_epi()
                                pending_epi = None
                            pends.append((e, kc, o))
                            if len(pends) > 2:
                                flush(pends.pop(0))
                            if (kc == max(1, nkc - 11) and idx + 1 < len(pairs)
                                    and pairs[idx + 1][0] != hl):
                                # next head's absorbed keys: independent PE
                                # work, early enough to hide the copy chain
                                prologue(pairs[idx + 1][0])
                        for p in pends:
                            flush(p)
                        pends = []

                        rbc = P2r.tile([128, 512], F32, tag="rbc")
                        if idx == len(pairs) - 1:
                            # final pair: row-sum as PE ones-matmuls into an
                            # all-partition PSUM (the PE idles here anyway);
                            # skips the tree + Pool reduce off the critical
                            # path into phase 3
                            rs2 = PSmm.tile([128, 512], F32, tag="mm")
                            for j, ej in enumerate(last_es):
                                nc.tensor.matmul(
                                    rs2[:, :], onesw[:, :], ej[:, :],
                                    start=(j == 0),
                                    stop=(j == len(last_es) - 1))
                            nc.vector.reciprocal(rbc[:, :], rs2[:, :])
                        else:
                            rem = [t for lvl in levels for t in lvl]
                            while len(rem) > 1:
                                a, b2 = rem.pop(0), rem.pop(0)
                                s = P2t.tile([128, 512], F16, tag="tsum")
                                nc.vector.tensor_add(s[:, :], a[:, :],
                                                     b2[:, :])
                                rem.append(s)
                            rsb = P2r.tile([128, 512], F32, tag="rsb")
                            nc.gpsimd.partition_all_reduce(
                                rsb[:, :], rem[0][:, :], 128,
                                bass_isa.ReduceOp.add)
                            nc.vector.reciprocal(rbc[:, :], rsb[:, :])

                        def make_epi(hl=hl, qs=qs, ov_ps=ov_ps, rbc=rbc):
                            def epi():
                                otmp = P2o.tile([128, 512], F32, tag="otmp")
                                nc.vector.tensor_mul(otmp[:, :],
                                                     ov_ps[:, :], rbc[:, :])
                                nc.gpsimd.tensor_copy(oh_hi[:, hl, qs],
                                                      otmp[:, :])
                                nc.vector.tensor_sub(oh_lo[:, hl, qs],
                                                     otmp[:, :],
                                                     oh_hi[:, hl, qs])
                            return epi

                        pending_epi = make_epi()
                    if pending_epi is not None:
                        pending_epi()
                        pending_epi = None

                # ------ phase 3: output projection (3-term fp8 DoubleRow) --
                # (ctx_hi+ctx_lo) @ (ow_hi+ow_lo), lo*lo dropped; products at
                # scale 16*512, descaled on the host. Weight-stationary: each
                # lhsT chunk sweeps all 4 query blocks (1 Ldweights per 4-8
                # matmuls), psums per query block held across the dc row.
                with tc.tile_pool(name="P3s", bufs=3) as P3s, \
                     tc.tile_pool(name="PS3", bufs=8, space="PSUM") as PS3:
                    # hp0 stages first: heads 0-1 are ready well before the
                    # last head's epilogue lands
                    stages = [(owgh_t, 0, (oh_hi, oh_lo)),
                              (owgl_t, 0, (oh_hi,)),
                              (owgh_t, 1, (oh_hi, oh_lo)),
                              (owgl_t, 1, (oh_hi,))]
                    for dc in range(NDC):
                        pss = []
                        for _ in range(NQB):
                            opps = PS3.tile([128, 512], F32, tag="op")
                            pss.append(opps)
                        for si, (wt, hp, cts) in enumerate(stages):
                            hs2 = bass.ds(2 * hp, 2)
                            for ci, ct in enumerate(cts):
                                for qb in range(NQB):
                                    nc.tensor.matmul(
                                        pss[qb][:, :],
                                        wt[:, hs2, bass.ts(dc, 128)],
                                        ct[:, hs2, bass.ds(qb * 512, 512)],
                                        start=(si == 0 and ci == 0),
                                        stop=(si == 3),
                                        perf_mode=DR)
                        st = P3s.tile([128, NQB, 512], BF16, tag="st")
                        for qb in range(NQB):
                            if (dc * NQB + qb) % 2 == 0:
                                nc.scalar.copy(st[:, qb, :], pss[qb][:, :])
                            else:
                                nc.vector.tensor_copy(st[:, qb, :],
                                                      pss[qb][:, :])
                            if dc >= NDC - 2 and qb == 1:
                                # final rows: ship the first half as soon as
                                # its copies land, shortening the drain
                                nc.sync.dma_start(
                                    outT[bass.ts(dc, 128), 0:1024],
                                    st[:, 0:2, :])
                        if dc >= NDC - 2:
                            nc.sync.dma_start(
                                outT[bass.ts(dc, 128), 1024:2048],
                                st[:, 2:4, :])
                        else:
                            # one batched DMA per dc row (fewer descriptors)
                            nc.sync.dma_start(outT[bass.ts(dc, 128), :],
                                              st[:, :, :])

    nc.compile()
    return nc


_NC_CACHE = None


def _get_nc():
    global _NC_CACHE
    if _NC_CACHE is None:
        _NC_CACHE = build_nc()
    return _NC_CACHE


def _host_prep(inputs):
    f32 = np.float32
    hs = np.asarray(inputs["hidden_states"], f32)
    qdw = np.asarray(inputs["q_down_w"], f32)
    qnw_full = np.asarray(inputs["q_up_nope_w"], f32)
    qrw_full = np.asarray(inputs["q_up_rope_w"], f32)
    kvdw = np.asarray(inputs["kv_down_w"], f32)
    krw = np.asarray(inputs["k_rope_w"], f32)
    wuk_full = np.asarray(inputs["w_uk"], f32)
    wuv_full = np.asarray(inputs["w_uv"], f32)
    ow = np.asarray(inputs["out_w"], f32)
    cos2, sin2 = _rope_tables()
    maskv = _masks()
    hsT8 = [_split8(hs[b].T, 16.0) for b in range(B)]
    kvdw8 = _split8(kvdw, 512.0)
    krw2 = np.concatenate([krw, krw], axis=1)            # [D, 128]
    krw8 = _split8(krw2, 512.0)
    wqn_full = qdw @ qnw_full                            # [D, H*NOPE]
    wqr_full = qdw @ qrw_full                            # [D, H*ROPE]
    in_maps = []
    for c in range(8):
        b, g = divmod(c, G)
        wqn8 = _split8(wqn_full[:, g * HPC * NOPE:(g + 1) * HPC * NOPE], 512.0)
        wqr8 = _split8(wqr_full[:, g * HPC * ROPE:(g + 1) * HPC * ROPE], 512.0)
        wukg = wuk_full[g * HPC * NOPE:(g + 1) * HPC * NOPE, :]
        wukT = np.ascontiguousarray(np.concatenate(
            [wukg[hl * NOPE:(hl + 1) * NOPE, :].T for hl in range(HPC)], 0))
        wuvg = wuv_full[g * HPC * VD:(g + 1) * HPC * VD, :]
        wuv4 = np.ascontiguousarray(wuvg.T)
        owgv = np.ascontiguousarray(ow[g * HPC * VD:(g + 1) * HPC * VD, :])
        owgh_v, owgl_v = _split8(owgv, 512.0)
        in_maps.append({
            "hsTh": hsT8[b][0], "hsTl": hsT8[b][1],
            "wqnh": wqn8[0], "wqnl": wqn8[1],
            "wqrh": wqr8[0], "wqrl": wqr8[1],
            "kvdwh": kvdw8[0], "kvdwl": kvdw8[1],
            "krwh": krw8[0], "krwl": krw8[1],
            "wukT": wukT,
            "wuv4": wuv4,
            "owgh": owgh_v, "owgl": owgl_v,
            "cos2d": cos2.astype(ml_dtypes.bfloat16),
            "sin2d": sin2.astype(ml_dtypes.bfloat16),
            "maskd": maskv.astype(np.float16),
        })
    return in_maps


def kernel(**inputs):
    nc = _get_nc()
    in_maps = _host_prep(inputs)
    res = run_bass_kernel_spmd(nc, in_maps, core_ids=list(range(8)))
    out = np.zeros((B, S, D), np.float32)
    for c in range(8):
        out[c // G] += res.results[c]["outT"].T.astype(np.float32)
    out *= np.float32(1.0 / SIG)
    out += np.asarray(inputs["out_b"], np.float32)[None, None, :]
    return out


# revision 102
# speedup vs baseline: 1.0023x; 1.0023x over previous
"""DeepSeek-V3 MLA attention kernel for 8 Trainium2 NeuronCores.

Problem: nn_DeepSeekV3_1Attention (B=2, S=2048, D=2048, H=16, NOPE=128,
ROPE=64, VD=128, QL=KVL=512), fp32 reference, causal.

Sharding: data-parallel over batch (2 groups of 4 cores) x tensor-parallel
over heads (4 heads per core). Each core computes its batch's shared
projections redundantly, runs MLA attention for its 4 heads, and produces
a partial out-projection (its heads' rows of out_w). Host sums the 4
partials per batch.

Phase 1 fuses the low-rank q path on the host (W_qn = q_down @ q_up_nope,
W_qr = q_down @ q_up_rope) so q_nope^T / q_rope^T / c_kv^T / k_rope^T all
come from one weight-stationary pass over hs^T: each 128-col weight chunk
sweeps all four 512-token blocks with a single Ldweights per lhsT (the PE
sequencer, not the array, limits fp8 DoubleRow throughput otherwise).
Projections stay at the fp8-pair scale (sigma = 16*512); the sigma^2 is
divided out inside the softmax exp, and the 1/sigma for the value path
rides the vabs copies.

Attention keeps sequence on the free dimension (scores^T[k, q]) with
exp(s - 2) in fp16, a multiplicative causal mask, the softmax denominator
as a pairwise fp16 tree on the DVE plus one cross-partition reduce on
Pool, and normalization deferred past the (linear) PV and out-projection.
"""

import numpy as np
import ml_dtypes

NF8 = ml_dtypes.float8_e4m3


def _split8(x, s):
    """x*s ~ hi + lo, both fp8e4m3 (shared power-of-2 scale s)."""
    xs = np.asarray(x, np.float32) * np.float32(s)
    hi = xs.astype(NF8)
    lo = (xs - hi.astype(np.float32)).astype(NF8)
    return np.ascontiguousarray(hi), np.ascontiguousarray(lo)

from concourse import bacc
import concourse.bass as bass
import concourse.bass_isa as bass_isa
import concourse.mybir as mybir
import concourse.tile as tile
from concourse.bass_utils import run_bass_kernel_spmd

F32 = mybir.dt.float32
F32R = mybir.dt.float32r
BF16 = mybir.dt.bfloat16
F16 = mybir.dt.float16
F8 = mybir.dt.float8e4
DR = mybir.MatmulPerfMode.DoubleRow
AF = mybir.ActivationFunctionType

B, S, D = 2, 2048, 2048
H = 16
NOPE, ROPE, VD = 128, 64, 128
QL, KVL = 512, 512
HPC = 4    # heads per core
G = 4      # cores per batch group
SCALE = float(1.0 / np.sqrt(np.float32(NOPE + ROPE)))
SIG = 16.0 * 512.0      # fp8-pair product scale carried by all projections
ESCALE = SCALE / (SIG * SIG)
EXPB = -2.0   # exp bias: e' = exp(s - 2) keeps row sums within fp16 range

ROPE_WAVELENGTH = 10000.0
ROPE_SCALE = 40.0
BETA_FAST, BETA_SLOW = 32.0, 1.0
OLD_CTX = 4096.0
MSCALE = 1.0
PI = 3.14159265358979

NDC = D // 128          # 16 d-chunks
NQLC = QL // 128        # 4 ql chunks
NKVC = KVL // 128       # 4 kv chunks
NKC = S // 128          # 16 key chunks
NQB = S // 512          # 4 query blocks


def _rope_tables():
    j = np.arange(0, ROPE, 2, dtype=np.float32) / ROPE
    freqs = (1.0 / (ROPE_WAVELENGTH ** j)).astype(np.float32)
    wavelengths = 2.0 * PI / freqs
    ramp = np.clip((wavelengths / OLD_CTX - BETA_SLOW) / (BETA_FAST - BETA_SLOW),
                   0.0, 1.0)
    scale = (1.0 - ramp) + ramp * ROPE_SCALE
    inv_freq = freqs / scale
    t = np.arange(S, dtype=np.float32)
    fr = t[:, None] * inv_freq[None, :]
    cos = (np.cos(fr) * MSCALE).astype(np.float32).T        # [32, S]
    sin = (np.sin(fr) * MSCALE).astype(np.float32).T
    cosT = np.concatenate([cos, cos], 0)                    # [64, S]
    sinT = np.concatenate([-sin, sin], 0)
    cos2 = np.ascontiguousarray(np.concatenate([cosT, cosT], 0))   # [128, S]
    sin2 = np.ascontiguousarray(np.concatenate([sinT, sinT], 0))
    return cos2, sin2


def _masks():
    # multiplicative 0/1 masks applied to exp(scores) on the diagonal chunks
    k = np.arange(128)[:, None]
    q = np.arange(512)[None, :]
    ms = []
    for m in range(4):
        allow = (k + m * 128) <= q
        ms.append(np.where(allow, np.float32(1.0), np.float32(0.0)))
    return np.ascontiguousarray(np.stack(ms, axis=1))    # [128, 4, 512]


def build_nc():
    nc = bacc.Bacc("TRN2", target_bir_lowering=False, debug=False,
                   enable_asserts=False, num_devices=8)

    hsTh = nc.dram_tensor("hsTh", [D, S], F8, kind="ExternalInput").ap()
    hsTl = nc.dram_tensor("hsTl", [D, S], F8, kind="ExternalInput").ap()
    wqnh = nc.dram_tensor("wqnh", [D, HPC * NOPE], F8, kind="ExternalInput").ap()
    wqnl = nc.dram_tensor("wqnl", [D, HPC * NOPE], F8, kind="ExternalInput").ap()
    wqrh = nc.dram_tensor("wqrh", [D, HPC * ROPE], F8, kind="ExternalInput").ap()
    wqrl = nc.dram_tensor("wqrl", [D, HPC * ROPE], F8, kind="ExternalInput").ap()
    kvdwh = nc.dram_tensor("kvdwh", [D, KVL], F8, kind="ExternalInput").ap()
    kvdwl = nc.dram_tensor("kvdwl", [D, KVL], F8, kind="ExternalInput").ap()
    krwh = nc.dram_tensor("krwh", [D, 2 * ROPE], F8, kind="ExternalInput").ap()
    krwl = nc.dram_tensor("krwl", [D, 2 * ROPE], F8, kind="ExternalInput").ap()
    wukT = nc.dram_tensor("wukT", [HPC * KVL, NOPE], F32R, kind="ExternalInput").ap()
    wuv4 = nc.dram_tensor("wuv4", [KVL, HPC * VD], F32R, kind="ExternalInput").ap()
    owgh = nc.dram_tensor("owgh", [HPC * VD, D], F8, kind="ExternalInput").ap()
    owgl = nc.dram_tensor("owgl", [HPC * VD, D], F8, kind="ExternalInput").ap()
    cos2d = nc.dram_tensor("cos2d", [2 * ROPE, S], BF16, kind="ExternalInput").ap()
    sin2d = nc.dram_tensor("sin2d", [2 * ROPE, S], BF16, kind="ExternalInput").ap()
    maskd = nc.dram_tensor("maskd", [128, 4, 512], F16, kind="ExternalInput").ap()
    outT = nc.dram_tensor("outT", [D, S], BF16, kind="ExternalOutput").ap()

    hsTh_r = hsTh.rearrange("(c p) s -> p c s", p=128)    # [128, 16, S]
    hsTl_r = hsTl.rearrange("(c p) s -> p c s", p=128)
    wqnh_r = wqnh.rearrange("(c p) q -> p c q", p=128)    # [128, 16, 512]
    wqnl_r = wqnl.rearrange("(c p) q -> p c q", p=128)
    wqrh_r = wqrh.rearrange("(c p) q -> p c q", p=128)    # [128, 16, 256]
    wqrl_r = wqrl.rearrange("(c p) q -> p c q", p=128)
    kvdwh_r = kvdwh.rearrange("(c p) q -> p c q", p=128)
    kvdwl_r = kvdwl.rearrange("(c p) q -> p c q", p=128)
    krwh_r = krwh.rearrange("(c p) q -> p c q", p=128)    # [128, 16, 128]
    krwl_r = krwl.rearrange("(c p) q -> p c q", p=128)
    wukT_r = wukT.rearrange("(c p) n -> p c n", p=128)    # [128, 16, 128]
    wuv4_r = wuv4.rearrange("(c p) v -> p c v", p=128)    # [128, 4, 512]
    owgh_r = owgh.rearrange("(h p) d -> p h d", p=128)    # [128, 4, D]
    owgl_r = owgl.rearrange("(h p) d -> p h d", p=128)

    with tile.TileContext(nc) as tc:
        with tc.tile_pool(name="A", bufs=1) as A:
            # phase-1 outputs, all at scale SIG (= 16*512); sequence on the
            # free dim so every downstream matmul contracts partitions
            c_kvT = A.tile([128, NKVC, S], F32R, tag="c_kvT")
            qnT = A.tile([128, HPC, S], BF16, tag="qnT")
            # two heads stacked per 128 partitions (psum-native layout)
            qrT2 = A.tile([128, HPC // 2, S], BF16, tag="qrT2")
            kr2 = A.tile([128, S], BF16, tag="kr2")     # k_rope duplicated 2x
            # phase-2 weights live here so their DMAs overlap phase 1
            # (a P2-scoped tile would inherit anti-deps on phase-1 SBUF)
            wukT_t = A.tile([128, HPC * NQLC, NOPE], F32R, tag="wukT")
            wuv4_t = A.tile([128, NKVC, HPC * VD], F32R, tag="wuv4")
            masks_t = A.tile([128, 4, 512], F16, tag="masks")

            # -------- phase 1: one weight-stationary pass over hs^T --------
            # 3-term fp8 DoubleRow over the D contraction; each 128-col
            # weight chunk sweeps all 4 s-blocks per Ldweights
            with tc.tile_pool(name="P1", bufs=1) as P1, \
                 tc.tile_pool(name="P1r", bufs=1) as P1r, \
                 tc.tile_pool(name="PS1", bufs=2, space="PSUM") as PS1:
                hsh_t = P1.tile([128, NDC, S], F8, tag="hsh")
                hsl_t = P1.tile([128, NDC, S], F8, tag="hsl")
                wqnh_t = P1.tile([128, NDC, HPC * NOPE], F8, tag="wqnh")
                wqnl_t = P1.tile([128, NDC, HPC * NOPE], F8, tag="wqnl")
                wqrh_t = P1.tile([128, NDC, HPC * ROPE], F8, tag="wqrh")
                wqrl_t = P1.tile([128, NDC, HPC * ROPE], F8, tag="wqrl")
                kvdwh_t = P1.tile([128, NDC, KVL], F8, tag="kvdwh")
                kvdwl_t = P1.tile([128, NDC, KVL], F8, tag="kvdwl")
                krwh_t = P1.tile([128, NDC, 2 * ROPE], F8, tag="krwh")
                krwl_t = P1.tile([128, NDC, 2 * ROPE], F8, tag="krwl")
                cos2_t = P1.tile([2 * ROPE, S], BF16, tag="cos2")
                sin2_t = P1.tile([2 * ROPE, S], BF16, tag="sin2")
                # DMA order tracks first use: a sliver of c_kv weights, two
                # hs chunks (first matmuls start ~4us in), the rest of the
                # weights, then the hs stream (its per-chunk DMA rate matches
                # PE consumption); q-path weights trail (first used 5
                # tile-cols in)
                # first matmul needs only these two transfers
                nc.sync.dma_start(kvdwh_t[:, 0:2, :], kvdwh_r[:, 0:2, :])
                nc.sync.dma_start(hsh_t[:, 0:2, :], hsTh_r[:, 0:2, :])
                nc.sync.dma_start(kvdwl_t[:, 0:2, :], kvdwl_r[:, 0:2, :])
                nc.sync.dma_start(hsl_t[:, 0:2, :], hsTl_r[:, 0:2, :])
                # kvdw slices ride just ahead of the hs chunks each dp needs
                for lo, hi in ((2, 4), (4, 8), (8, NDC)):
                    nc.sync.dma_start(kvdwh_t[:, lo:hi, :],
                                      kvdwh_r[:, lo:hi, :])
                    nc.sync.dma_start(kvdwl_t[:, lo:hi, :],
                                      kvdwl_r[:, lo:hi, :])
                    for c in range(lo, hi):
                        nc.sync.dma_start(hsh_t[:, c, :], hsTh_r[:, c, :])
                        nc.sync.dma_start(hsl_t[:, c, :], hsTl_r[:, c, :])
                nc.sync.dma_start(krwh_t[:, :, :], krwh_r[:, :, :])
                nc.sync.dma_start(krwl_t[:, :, :], krwl_r[:, :, :])
                nc.sync.dma_start(cos2_t[:, :], cos2d[:, :])
                nc.sync.dma_start(sin2_t[:, :], sin2d[:, :])
                for t, r in [(wqnh_t, wqnh_r), (wqnl_t, wqnl_r),
                             (wqrh_t, wqrh_r), (wqrl_t, wqrl_r)]:
                    nc.sync.dma_start(t[:, :, :], r[:, :, :])
                nc.sync.dma_start(masks_t[:, :, :], maskd[:, :, :])
                nc.sync.dma_start(wukT_t[:, :, :], wukT_r[:, :, :])
                nc.sync.dma_start(wuv4_t[:, :, :], wuv4_r[:, :, :])

                cp_rot = [0]

                def rot_copy(dst, src):
                    # PSUM->SBUF: DVE/Act only (Pool cannot access PSUM)
                    if cp_rot[0] % 2 == 0:
                        nc.scalar.copy(dst, src)
                    else:
                        nc.vector.tensor_copy(dst, src)
                    cp_rot[0] += 1

                def run_tilecols(group):
                    # dp-major across the group's tile-cols: the first two
                    # run interleaved so ~21us of PE demand rides the
                    # ~28us hs DMA stream instead of ~10us
                    pss = []
                    for _ in group:
                        ps1t = PS1.tile([128, NQB, 512], F32, tag="p1ps")
                        pss.append(ps1t)
                    for dp in range(NDC // 2):
                        d2 = bass.ds(2 * dp, 2)
                        for ci, (wh_t, wl_t, cs, emit) in enumerate(group):
                            terms = [(wh_t, hsh_t), (wl_t, hsh_t),
                                     (wh_t, hsl_t)]
                            for ti, (wt, ht) in enumerate(terms):
                                for sb in range(NQB):
                                    nc.tensor.matmul(
                                        pss[ci][:, sb, :], wt[:, d2, cs],
                                        ht[:, d2, bass.ds(sb * 512, 512)],
                                        start=(dp == 0 and ti == 0),
                                        stop=(dp == NDC // 2 - 1 and ti == 2),
                                        perf_mode=DR)
                    for ci, (wh_t, wl_t, cs, emit) in enumerate(group):
                        emit(pss[ci])

                def emit_copy(dst):
                    def e(ps):
                        rot_copy(dst, ps[:, :, :])
                    return e

                def emit_rope(dst):
                    def e(ps):
                        # rope via swap trick: out = raw*cos + swap(raw)*sin
                        # (sin table carries the sign). One copy drains the
                        # PSUM strip immediately; swaps then run SBUF->SBUF
                        # off the PE-critical path.
                        t1 = P1r.tile([128, NQB * 512], BF16, tag="t1")
                        rot_copy(t1[:, :], ps[:, :, :])
                        sw = P1r.tile([128, NQB * 512], BF16, tag="sw")
                        nc.scalar.copy(sw[0:32, :], t1[32:64, :])
                        nc.vector.tensor_copy(sw[32:64, :], t1[0:32, :])
                        nc.scalar.copy(sw[64:96, :], t1[96:128, :])
                        nc.vector.tensor_copy(sw[96:128, :], t1[64:96, :])
                        nc.vector.tensor_mul(t1[:, :], t1[:, :], cos2_t[:, :])
                        nc.vector.tensor_mul(sw[:, :], sw[:, :], sin2_t[:, :])
                        nc.vector.tensor_add(dst, t1[:, :], sw[:, :])
                    return e

                cols = []
                for qlc in range(NKVC):
                    cols.append((kvdwh_t, kvdwl_t, bass.ts(qlc, 128),
                                 emit_copy(c_kvT[:, qlc, :])))
                cols.append((krwh_t, krwl_t, bass.ds(0, 128),
                             emit_rope(kr2[:, :])))
                cols.append((wqnh_t, wqnl_t, bass.ts(0, 128),
                             emit_copy(qnT[:, 0, :])))
                cols.append((wqrh_t, wqrl_t, bass.ts(0, 128),
                             emit_rope(qrT2[:, 0, :])))
                cols.append((wqnh_t, wqnl_t, bass.ts(1, 128),
                             emit_copy(qnT[:, 1, :])))
                cols.append((wqnh_t, wqnl_t, bass.ts(2, 128),
                             emit_copy(qnT[:, 2, :])))
                cols.append((wqrh_t, wqrl_t, bass.ts(1, 128),
                             emit_rope(qrT2[:, 1, :])))
                cols.append((wqnh_t, wqnl_t, bass.ts(3, 128),
                             emit_copy(qnT[:, 3, :])))
                run_tilecols(cols[0:2])
                for col in cols[2:]:
                    run_tilecols([col])

            # per-head context as residual fp8 pair at scale 16 (the 16
            # rides the vabs copies; phase 3 descales on the host)
            with tc.tile_pool(name="A2", bufs=1) as A2:
                oh_hi = A2.tile([128, HPC, S], F8, tag="oh_hi")
                oh_lo = A2.tile([128, HPC, S], F8, tag="oh_lo")
                # phase-3 weights: DMA overlaps the attention phase
                owgh_t = A2.tile([128, HPC, D], F8, tag="owgh")
                owgl_t = A2.tile([128, HPC, D], F8, tag="owgl")
                for hl in range(HPC):
                    nc.sync.dma_start(owgh_t[:, hl, :], owgh_r[:, hl, :])
                    nc.sync.dma_start(owgl_t[:, hl, :], owgl_r[:, hl, :])

                # -------- phase 2: per-head attention --------
                with tc.tile_pool(name="P2", bufs=1) as P2, \
                     tc.tile_pool(name="P2q2", bufs=2) as P2q2, \
                     tc.tile_pool(name="P2v", bufs=1) as P2v, \
                     tc.tile_pool(name="P2e", bufs=6) as P2e, \
                     tc.tile_pool(name="P2o", bufs=2) as P2o, \
                     tc.tile_pool(name="P2r", bufs=1) as P2r, \
                     tc.tile_pool(name="P2t", bufs=6) as P2t, \
                     tc.tile_pool(name="PSmm", bufs=5, space="PSUM") as PSmm, \
                     tc.tile_pool(name="PSov", bufs=3, space="PSUM") as PSov:
                    biasb = P2.tile([128, 1], F32, tag="expb")
                    nc.vector.memset(biasb[:, :], EXPB)

                    # absorbed values for all 4 heads in one N=512 pass:
                    # vabs = c_kv @ w_uv^T, x(16/SIG) in the copies
                    vabs4 = P2v.tile([128, NKC, HPC * VD], F16, tag="vabs")
                    VSC = 16.0 / SIG
                    for kc in range(NKC):
                        ps4 = PSmm.tile([128, HPC * VD], F32, tag="mm")
                        for kvc in range(NKVC):
                            nc.tensor.matmul(
                                ps4[:, :],
                                c_kvT[:, kvc, bass.ts(kc, 128)],
                                wuv4_t[:, kvc, :],
                                start=(kvc == 0), stop=(kvc == NKVC - 1))
                        if kc % 2 == 0:
                            nc.vector.tensor_scalar_mul(vabs4[:, kc, :],
                                                        ps4[:, :], VSC)
                        else:
                            nc.scalar.activation(vabs4[:, kc, :], ps4[:, :],
                                                 AF.Copy, scale=VSC)

                    self_ka = [None]   # current head's absorbed keys
                    # wide f16 ones: the final pair's row-sum runs as PE
                    # matmuls (broadcast across partitions), shortening the
                    # serial epilogue chain that gates phase 3
                    onesw = P2.tile([128, NOPE], F16, tag="onesw")
                    nc.vector.memset(onesw[:, :], 1.0)

                    def prologue(hl):
                        """absorbed keys k_abs = w_uk_h @ c_kv^T for one head
                        (contracting scores over NOPE=128 instead of KVL)."""
                        kabs = P2q2.tile([128, S], BF16, tag="kabs")
                        for b4 in range(NQB):
                            s4 = bass.ds(b4 * 512, 512)
                            ps3 = PSmm.tile([128, 512], F32, tag="mm")
                            for latc in range(NQLC):
                                nc.tensor.matmul(
                                    ps3[:, :],
                                    wukT_t[:, hl * NQLC + latc, :],
                                    c_kvT[:, latc, s4],
                                    start=(latc == 0), stop=(latc == NQLC - 1))
                            if b4 % 2 == 0:
                                nc.vector.tensor_copy(kabs[:, s4], ps3[:, :])
                            else:
                                nc.scalar.copy(kabs[:, s4], ps3[:, :])
                        self_ka[0] = kabs
                        return kabs

                    # last head runs its big qb=3 block early so that
                    # epilogue hides under the small qb=0 block, shortening
                    # the stall before phase 3's hp1 stages
                    pairs = [(hl, qb) for hl in range(HPC)
                             for qb in (1, 2, 3, 0)]
                    kabs = prologue(0)
                    pending_epi = None   # deferred out_v + normalize
                    e_allocs = [0]       # P2e allocation counter (uninit guard)

                    for idx, (hl, qb) in enumerate(pairs):
                        qs = bass.ds(qb * 512, 512)
                        nkc = 4 * qb + 4
                        kabs = self_ka[0]
                        half = 64 * (hl % 2)

                        ov_ps = PSov.tile([128, 512], F32, tag="ov")
                        # softmax denominator: pairwise fp16 tree on the DVE,
                        # then one cross-partition reduce on Pool
                        levels = [[] for _ in range(6)]
                        last_es = []

                        def tree_push(t, lv=0):
                            if levels[lv]:
                                a = levels[lv].pop()
                                s = P2t.tile([128, 512], F16, tag="tsum")
                                nc.vector.tensor_add(s[:, :], a[:, :], t[:, :])
                                tree_push(s, lv + 1)
                            else:
                                levels[lv].append(t)

                        pends = []   # deferred exp tiles for PE pipelining

                        def flush(pend, ov_ps=ov_ps, nkc=nkc, hl=hl):
                            e, kc, o = pend
                            nc.tensor.matmul(
                                ov_ps[:, o:512],
                                vabs4[:, kc, bass.ds(hl * VD, VD)],
                                e[:, o:512],
                                start=(kc == 0), stop=(kc == nkc - 1))

                        for kc in range(nkc):
                            # diagonal chunks: exact causal width
                            m = kc - 4 * qb
                            o = 0 if m < 0 else m * 128
                            ps_s = PSmm.tile([128, 512], F32, tag="mm")
                            nc.tensor.matmul(
                                ps_s[:, o:512],
                                kabs[:, bass.ts(kc, 128)],
                                qnT[:, hl, bass.ds(qb * 512 + o, 512 - o)],
                                start=True, stop=False)
                            nc.tensor.matmul(
                                ps_s[:, o:512],
                                kr2[half:half + 64, bass.ts(kc, 128)],
                                qrT2[half:half + 64, hl // 2,
                                     bass.ds(qb * 512 + o, 512 - o)],
                                start=False, stop=True)
                            e = P2e.tile([128, 512], F16, tag="exp")
                            if m >= 0 and o > 0 and e_allocs[0] < 12:
                                # first pool rotations: [0:o) is uninit SBUF;
                                # zero it so the full-width mask mul below
                                # never reads garbage bits (NaN-safe on hw)
                                nc.gpsimd.memset(e[:, 0:o], 0.0)
                            e_allocs[0] += 1
                            nc.scalar.activation(e[:, o:512], ps_s[:, o:512],
                                                 AF.Exp, scale=ESCALE,
                                                 bias=biasb[:, :])
                            if m >= 0:
                                # causal mask, full width: also zeroes the
                                # stale [0:o) region for the denominator tree
                                nc.vector.tensor_mul(
                                    e[:, :], e[:, :], masks_t[:, m, :])
                            if idx == len(pairs) - 1:
                                last_es.append(e)
                            else:
                                tree_push(e)
                            if kc == (3 if nkc == 4 else 5) and pending_epi is not None:
                                # previous pair's out_v runs two score-blocks
                                # into this pair, hiding its ctx copy latency
                                pending_epi()
                                pending_epi = None
                            pends.append((e, kc, o))
                            if len(pends) > 2:
                                flush(pends.pop(0))
                            if (kc == max(1, nkc - 11) and idx + 1 < len(pairs)
                                    and pairs[idx + 1][0] != hl):
                                # next head's absorbed keys: independent PE
                                # work, early enough to hide the copy chain
                                prologue(pairs[idx + 1][0])
                        for p in pends:
                            flush(p)
                        pends = []

                        rbc = P2r.tile([128, 512], F32, tag="rbc")
                        if idx == len(pairs) - 1:
                            # final pair: row-sum as PE ones-matmuls into an
                            # all-partition PSUM (the PE idles here anyway);
                            # skips the tree + Pool reduce off the critical
                            # path into phase 3
                            rs2 = PSmm.tile([128, 512], F32, tag="mm")
                            for j, ej in enumerate(last_es):
                                nc.tensor.matmul(
                                    rs2[:, :], onesw[:, :], ej[:, :],
                                    start=(j == 0),
                                    stop=(j == len(last_es) - 1))
                            nc.vector.reciprocal(rbc[:, :], rs2[:, :])
                        else:
                            rem = [t for lvl in levels for t in lvl]
                            while len(rem) > 1:
                                a, b2 = rem.pop(0), rem.pop(0)
                                s = P2t.tile([128, 512], F16, tag="tsum")
                                nc.vector.tensor_add(s[:, :], a[:, :],
                                                     b2[:, :])
                                rem.append(s)
                            rsb = P2r.tile([128, 512], F32, tag="rsb")
                            nc.gpsimd.partition_all_reduce(
                                rsb[:, :], rem[0][:, :], 128,
                                bass_isa.ReduceOp.add)
                            nc.vector.reciprocal(rbc[:, :], rsb[:, :])

                        def make_epi(hl=hl, qs=qs, ov_ps=ov_ps, rbc=rbc):
                            def epi():
                                otmp = P2o.tile([128, 512], F32, tag="otmp")
                                nc.vector.tensor_mul(otmp[:, :],
                                                     ov_ps[:, :], rbc[:, :])
                                nc.gpsimd.tensor_copy(oh_hi[:, hl, qs],
                                                      otmp[:, :])
                                nc.vector.tensor_sub(oh_lo[:, hl, qs],
                                                     otmp[:, :],
                                                     oh_hi[:, hl, qs])
                            return epi

                        pending_epi = make_epi()
                    if pending_epi is not None:
                        pending_epi()
                        pending_epi = None

                # ------ phase 3: output projection (3-term fp8 DoubleRow) --
                # (ctx_hi+ctx_lo) @ (ow_hi+ow_lo), lo*lo dropped; products at
                # scale 16*512, descaled on the host. Weight-stationary: each
                # lhsT chunk sweeps all 4 query blocks (1 Ldweights per 4-8
                # matmuls), psums per query block held across the dc row.
                with tc.tile_pool(name="P3s", bufs=3) as P3s, \
                     tc.tile_pool(name="PS3", bufs=8, space="PSUM") as PS3:
                    # hp0 stages first: heads 0-1 are ready well before the
                    # last head's epilogue lands
                    stages = [(owgh_t, 0, (oh_hi, oh_lo)),
                              (owgl_t, 0, (oh_hi,)),
                              (owgh_t, 1, (oh_hi, oh_lo)),
                              (owgl_t, 1, (oh_hi,))]
                    for dc in range(NDC):
                        pss = []
                        for _ in range(NQB):
                            opps = PS3.tile([128, 512], F32, tag="op")
                            pss.append(opps)
                        for si, (wt, hp, cts) in enumerate(stages):
                            hs2 = bass.ds(2 * hp, 2)
                            for ci, ct in enumerate(cts):
                                for qb in range(NQB):
                                    nc.tensor.matmul(
                                        pss[qb][:, :],
                                        wt[:, hs2, bass.ts(dc, 128)],
                                        ct[:, hs2, bass.ds(qb * 512, 512)],
                                        start=(si == 0 and ci == 0),
                                        stop=(si == 3),
                                        perf_mode=DR)
                        st = P3s.tile([128, NQB, 512], BF16, tag="st")
                        for qb in range(NQB):
                            if (dc * NQB + qb) % 2 == 0:
                                nc.scalar.copy(st[:, qb, :], pss[qb][:, :])
                            else:
                                nc.vector.tensor_copy(st[:, qb, :],
                                                      pss[qb][:, :])
                            if dc >= NDC - 2 and qb == 1:
                                # final rows: ship the first half as soon as
                                # its copies land, shortening the drain
                                nc.sync.dma_start(
                                    outT[bass.ts(dc, 128), 0:1024],
                                    st[:, 0:2, :])
                        if dc >= NDC - 2:
                            nc.sync.dma_start(
                                outT[bass.ts(dc, 128), 1024:2048],
                                st[:, 2:4, :])
                        else:
                            # one batched DMA per dc row (fewer descriptors)
                            nc.sync.dma_start(outT[bass.ts(dc, 128), :],
                                              st[:, :, :])

    nc.compile()
    return nc


_NC_CACHE = None


def _get_nc():
    global _NC_CACHE
    if _NC_CACHE is None:
        _NC_CACHE = build_nc()
    return _NC_CACHE


def _host_prep(inputs):
    f32 = np.float32
    hs = np.asarray(inputs["hidden_states"], f32)
    qdw = np.asarray(inputs["q_down_w"], f32)
    qnw_full = np.asarray(inputs["q_up_nope_w"], f32)
    qrw_full = np.asarray(inputs["q_up_rope_w"], f32)
    kvdw = np.asarray(inputs["kv_down_w"], f32)
    krw = np.asarray(inputs["k_rope_w"], f32)
    wuk_full = np.asarray(inputs["w_uk"], f32)
    wuv_full = np.asarray(inputs["w_uv"], f32)
    ow = np.asarray(inputs["out_w"], f32)
    cos2, sin2 = _rope_tables()
    maskv = _masks()
    hsT8 = [_split8(hs[b].T, 16.0) for b in range(B)]
    kvdw8 = _split8(kvdw, 512.0)
    krw2 = np.concatenate([krw, krw], axis=1)            # [D, 128]
    krw8 = _split8(krw2, 512.0)
    wqn_full = qdw @ qnw_full                            # [D, H*NOPE]
    wqr_full = qdw @ qrw_full                            # [D, H*ROPE]
    in_maps = []
    for c in range(8):
        b, g = divmod(c, G)
        wqn8 = _split8(wqn_full[:, g * HPC * NOPE:(g + 1) * HPC * NOPE], 512.0)
        wqr8 = _split8(wqr_full[:, g * HPC * ROPE:(g + 1) * HPC * ROPE], 512.0)
        wukg = wuk_full[g * HPC * NOPE:(g + 1) * HPC * NOPE, :]
        wukT = np.ascontiguousarray(np.concatenate(
            [wukg[hl * NOPE:(hl + 1) * NOPE, :].T for hl in range(HPC)], 0))
        wuvg = wuv_full[g * HPC * VD:(g + 1) * HPC * VD, :]
        wuv4 = np.ascontiguousarray(wuvg.T)
        owgv = np.ascontiguousarray(ow[g * HPC * VD:(g + 1) * HPC * VD, :])
        owgh_v, owgl_v = _split8(owgv, 512.0)
        in_maps.append({
            "hsTh": hsT8[b][0], "hsTl": hsT8[b][1],
            "wqnh": wqn8[0], "wqnl": wqn8[1],
            "wqrh": wqr8[0], "wqrl": wqr8[1],
            "kvdwh": kvdw8[0], "kvdwl": kvdw8[1],
            "krwh": krw8[0], "krwl": krw8[1],
            "wukT": wukT,
            "wuv4": wuv4,
            "owgh": owgh_v, "owgl": owgl_v,
            "cos2d": cos2.astype(ml_dtypes.bfloat16),
            "sin2d": sin2.astype(ml_dtypes.bfloat16),
            "maskd": maskv.astype(np.float16),
        })
    return in_maps


def kernel(**inputs):
    nc = _get_nc()
    in_maps = _host_prep(inputs)
    res = run_bass_kernel_spmd(nc, in_maps, core_ids=list(range(8)))
    out = np.zeros((B, S, D), np.float32)
    for c in range(8):
        out[c // G] += res.results[c]["outT"].T.astype(np.float32)
    out *= np.float32(1.0 / SIG)
    out += np.asarray(inputs["out_b"], np.float32)[None, None, :]
    return out
